# revision 1
# baseline (speedup 1.0000x reference)
"""Trainium2 Bass kernel for the BiAttention problem.

Math (per batch b, L=1024, D=256):
  s0[i] = sum_d c[i,d] * c_weight[d]          (per-row constant)
  s1[j] = sum_d c[j,d] * q_weight[d]
  s2[i,j] = sum_d (c[i,d]*cqw[d]) * q[j,d]
  S = s0 + s1 + s2 (+bias; bias is a scalar so it cancels in both softmaxes)
  S1 = softmax_j(S)         (s0 cancels: S1 = exp(s2+s1)/rowsum)
  C2Q = S1 @ q
  S2[b,j,i] = softmax over b of S[b,i,j]  (cross-batch -> AllReduce of exp-sums)
  Q2C = S1 @ (S2 @ c)       (re-associated from (S1@S2)@c: saves 2x flops)
  out = concat(c, C2Q, c*C2Q, c*Q2C) on axis 0.

Sharding: batch B=16 over 8 cores (2 per core).  The only cross-core
data is Z[i,j] = sum_b exp(S[b,i,j]) -> one bf16 [1024,1024] AllReduce.

Dtype strategy: PE matmuls run as float32r (full-rate fp32, ~1e-4 rel
err) for everything feeding the exponentials; the S1-weighted GEMMs
(C2Q / Q2C) use bf16 operands; the cross-batch softmax denominator is
AllReduced in bf16.  Measured absmax-relative error ~1.4e-3.

Host does only O(B*L*D) prep: the two GEMV bias vectors s0/s1, slicing,
and the output concat (out[0:B] is the unchanged input c).
"""

import sys

import numpy as np

for _p in ("/opt/trn_rl_repo",):
    if _p not in sys.path:
        sys.path.insert(0, _p)

import concourse.bacc as bacc
import concourse.bass as bass
import concourse.mybir as mybir
import concourse.tile as tile
from concourse.bass_utils import run_bass_kernel_spmd
from concourse.masks import make_identity

F32 = mybir.dt.float32
BF16 = mybir.dt.bfloat16
AF = mybir.ActivationFunctionType
ALU = mybir.AluOpType
F32R = mybir.dt.float32r


def _r(ap):
    """Bitcast an fp32 AP to float32r: same bytes, PE runs 1 cycle/row
    (vs 4 for strict fp32) when the moving free dim is >= 256."""
    return ap.bitcast(F32R)

B, L, D = 16, 1024, 256
NCORES = 8
BPC = B // NCORES  # batches per core
P = 128
LB = L // P   # 8 L-blocks
DB = D // P   # 2 D-chunks

_CACHE = {}


def _build_nc():
    nc = bacc.Bacc(
        "TRN2",
        target_bir_lowering=False,
        debug=False,
        num_devices=NCORES,
    )

    # ---- kernel I/O ----
    c2 = nc.dram_tensor("c2", [BPC, L, D], F32, kind="ExternalInput")
    q2 = nc.dram_tensor("q2", [BPC, L, D], F32, kind="ExternalInput")
    s0c_d = nc.dram_tensor("s0c", [BPC, P, LB], F32, kind="ExternalInput")
    s1c_d = nc.dram_tensor("s1c", [BPC, P, LB], F32, kind="ExternalInput")
    s1r_d = nc.dram_tensor("s1r", [BPC, L], F32, kind="ExternalInput")
    cqw_d = nc.dram_tensor("cqw", [P, DB], F32, kind="ExternalInput")

    o_c2q = nc.dram_tensor("o_c2q", [BPC, L, D], F32, kind="ExternalOutput")
    o_cc2q = nc.dram_tensor("o_cc2q", [BPC, L, D], F32, kind="ExternalOutput")
    o_cq2c = nc.dram_tensor("o_cq2c", [BPC, L, D], F32, kind="ExternalOutput")

    rg = [list(range(NCORES))]

    with tile.TileContext(nc) as tc:
        with (
            tc.tile_pool(name="dram", bufs=1, space="DRAM") as dram,
            tc.tile_pool(name="small", bufs=1) as small,
            tc.tile_pool(name="cnat", bufs=1) as cnatp,
            tc.tile_pool(name="qnat", bufs=1) as qnatp,
            tc.tile_pool(name="big", bufs=8) as bigp,
            tc.tile_pool(name="Ep", bufs=16) as Ep,
            tc.tile_pool(name="E1Tp", bufs=16) as E1Tp,
            tc.tile_pool(name="Wp", bufs=16) as Wp,
            tc.tile_pool(name="st", bufs=2) as stp,
            tc.tile_pool(name="psT", bufs=2, space="PSUM") as psT,
            tc.tile_pool(name="psV", bufs=2, space="PSUM") as psV,
            tc.tile_pool(name="psS", bufs=2, space="PSUM") as psS,
        ):
            zin = dram.tile([L, L], BF16, name="zin")
            zout = dram.tile([L, L], BF16, name="zout", addr_space="Shared")

            # ---- constants / small vectors ----
            ident = small.tile([P, P], F32, name="ident")
            make_identity(nc, ident)
            identr = small.tile([P, P], F32, name="identr")
            nc.scalar.activation(_r(identr[:]), ident[:], AF.Copy)
            ones0 = small.tile([1, P], F32, name="ones0")
            nc.gpsimd.memset(ones0[:], 1.0)
            ones1 = small.tile([1, P], F32, name="ones1")
            nc.scalar.activation(_r(ones1[:]), ones0[:], AF.Copy)
            cqw = small.tile([P, DB], F32, name="cqw")
            nc.sync.dma_start(cqw[:], cqw_d[:, :])
            s0c = [small.tile([P, LB], F32, name=f"s0c{b}") for b in range(BPC)]
            s1c = [small.tile([P, LB], F32, name=f"s1c{b}") for b in range(BPC)]
            s1r = small.tile([1, BPC * L], F32, name="s1r")
            nc.sync.dma_start(_r(s1r[:]), _r(s1r_d.rearrange("b l -> (b l)")[None, :]))
            for b in range(BPC):
                nc.sync.dma_start(s0c[b][:], s0c_d[b])
                nc.sync.dma_start(s1c[b][:], s1c_d[b])

            # softmax-normalization scratch: rowsum of E, exp(s0), 1/D1
            rsE = [small.tile([P, LB], F32, name=f"rsE{b}") for b in range(BPC)]
            es0 = [small.tile([P, LB], F32, name=f"es0{b}") for b in range(BPC)]
            rrs = [small.tile([P, LB], F32, name=f"rrs{b}") for b in range(BPC)]
            rD1 = [small.tile([P, LB], F32, name=f"rD1{b}") for b in range(BPC)]

            # ---- bulk input loads (natural layout, [128, LB, D]) ----
            cnat, qnat, qnatb = [], [], []
            for b in range(BPC):
                ct = cnatp.tile([P, LB, D], F32, name=f"cnat{b}")
                c_src = c2[b].rearrange("(m p) d -> p m d", p=P)
                for h in range(4):
                    nc.sync.dma_start(
                        _r(ct[:, 2 * h:2 * h + 2]), _r(c_src[:, 2 * h:2 * h + 2])
                    )
                cnat.append(ct)
                qt = qnatp.tile([P, LB, D], F32, name=f"qnat{b}")
                q_src = q2[b].rearrange("(m p) d -> p m d", p=P)
                for h in range(4):
                    nc.sync.dma_start(
                        _r(qt[:, 2 * h:2 * h + 2]), _r(q_src[:, 2 * h:2 * h + 2])
                    )
                qnat.append(qt)
                qb = qnatp.tile([P, LB, D], BF16, name=f"qnatb{b}", tag="qb")
                nc.gpsimd.dma_start(
                    qb[:], q2[b].rearrange("(m p) d -> p m d", p=P)
                )
                qnatb.append(qb)

            # ---- transposed layouts via PE transpose ----
            # AT[b][t] = (c * cqw)^T chunk  [128(d), 1024(i)]
            # qT[b][t] = q^T chunk          [128(d), 1024(j)]
            AT = [[None] * DB for _ in range(BPC)]
            qT = [[None] * DB for _ in range(BPC)]
            for b in range(BPC):
                for t in range(DB):
                    AT[b][t] = bigp.tile([P, L], F32, name=f"AT{b}_{t}", tag="big")
                    qT[b][t] = bigp.tile([P, L], F32, name=f"qT{b}_{t}", tag="big")
                for m in range(LB):
                    for t in range(DB):
                        pt = psT.tile([P, P], F32, name="pt", tag="pt")
                        nc.tensor.transpose(
                            _r(pt[:]), _r(cnat[b][:, m, t * P:(t + 1) * P]),
                            _r(identr[:]),
                        )
                        # evacuate with the cq_weight scale fused in;
                        # alternate ACT/DVE so neither gates the AR start
                        if t == 0:
                            nc.scalar.activation(
                                _r(AT[b][t][:, m * P:(m + 1) * P]), pt[:],
                                AF.Copy, bias=0.0, scale=cqw[:, t:t + 1],
                            )
                        else:
                            nc.vector.tensor_scalar(
                                out=_r(AT[b][t][:, m * P:(m + 1) * P]),
                                in0=pt[:], scalar1=cqw[:, t:t + 1],
                                scalar2=None, op0=ALU.mult,
                            )
                        pt2 = psT.tile([P, P], F32, name="pt2", tag="pt")
                        nc.tensor.transpose(
                            _r(pt2[:]), _r(qnat[b][:, m, t * P:(t + 1) * P]),
                            _r(identr[:]),
                        )
                        if t == 0:
                            nc.vector.tensor_copy(
                                out=_r(qT[b][t][:, m * P:(m + 1) * P]),
                                in_=pt2[:],
                            )
                        else:
                            nc.scalar.activation(
                                _r(qT[b][t][:, m * P:(m + 1) * P]), pt2[:],
                                AF.Copy,
                            )

            # ---- phase 1: V = s2 + s1 (rank-1), E = exp(V + s0); Zpart to DRAM
            E = [[None] * LB for _ in range(BPC)]
            for b in range(BPC):
                for m in range(LB):
                    pv = psV.tile([P, L], F32, name="pv", tag="pv")
                    for n in range(2):
                        sl = slice(n * 512, (n + 1) * 512)
                        nc.tensor.matmul(
                            pv[:, sl], _r(AT[b][0][:, m * P:(m + 1) * P]),
                            _r(qT[b][0][:, sl]), start=True, stop=False,
                        )
                        nc.tensor.matmul(
                            pv[:, sl], _r(AT[b][1][:, m * P:(m + 1) * P]),
                            _r(qT[b][1][:, sl]), start=False, stop=False,
                        )
                        nc.tensor.matmul(
                            pv[:, sl], _r(ones1[0:1, :]),
                            _r(s1r[0:1, b * L + n * 512: b * L + (n + 1) * 512]),
                            start=False, stop=True,
                        )
                    E[b][m] = Ep.tile([P, L], F32, name=f"E{b}_{m}", tag="E")
                    nc.scalar.activation(
                        _r(E[b][m][:]), pv[:], AF.Exp,
                        bias=s0c[b][:, m:m + 1],
                        accum_out=rsE[b][:, m:m + 1],
                    )
                    nc.gpsimd.dma_start(
                        zin[m * P:(m + 1) * P, :], E[b][m][:],
                        accum_op=(ALU.bypass if b == 0 else ALU.add),
                    )
                # per-batch normalization vector 1/D1 = exp(s0)/rowsum(E)
                nc.scalar.activation(es0[b][:], s0c[b][:], AF.Exp)
                nc.vector.reciprocal_approx_fast(out=rrs[b][:], in_=rsE[b][:])
                nc.vector.tensor_mul(rD1[b][:], rrs[b][:], es0[b][:])

            # ---- cross-batch softmax denominator AllReduce ----
            nc.gpsimd.collective_compute(
                "AllReduce", ALU.add, replica_groups=rg,
                ins=[zin.opt()], outs=[zout.opt()],
            )

            # ---- phase 2a: VT = s2^T, E1T = exp(VT + s1) ----
            E1T = [[None] * LB for _ in range(BPC)]
            for b in range(BPC):
                for jm in range(LB):
                    pv = psV.tile([P, L], F32, name="pvt", tag="pv")
                    for n in range(2):
                        sl = slice(n * 512, (n + 1) * 512)
                        nc.tensor.matmul(
                            pv[:, sl], _r(qT[b][0][:, jm * P:(jm + 1) * P]),
                            _r(AT[b][0][:, sl]), start=True, stop=False,
                        )
                        nc.tensor.matmul(
                            pv[:, sl], _r(qT[b][1][:, jm * P:(jm + 1) * P]),
                            _r(AT[b][1][:, sl]), start=False, stop=True,
                        )
                    E1T[b][jm] = E1Tp.tile([P, L], BF16, name=f"E1T{b}_{jm}", tag="E1T")
                    nc.scalar.activation(
                        E1T[b][jm][:], pv[:], AF.Exp, bias=s1c[b][:, jm:jm + 1]
                    )

            # ---- phase 2b: C2Q = (E1T^T @ q) * rD1 ; outputs C2Q, c*C2Q ----
            for b in range(BPC):
                for m in range(LB):
                    ps = psS.tile([P, D], F32, name="psc", tag="ps")
                    for jk in range(LB):
                        nc.tensor.matmul(
                            ps[:], E1T[b][jk][:, m * P:(m + 1) * P],
                            qnatb[b][:, jk, :],
                            start=(jk == 0), stop=(jk == LB - 1),
                        )
                    c2qt = stp.tile([P, D], F32, name="c2qt", tag="c2q")
                    nc.vector.tensor_scalar(
                        out=c2qt[:], in0=ps[:], scalar1=rD1[b][:, m:m + 1],
                        scalar2=None, op0=ALU.mult,
                    )
                    nc.sync.dma_start(o_c2q[b, m * P:(m + 1) * P, :], c2qt[:])
                    cxt = stp.tile([P, D], F32, name="cxt", tag="cx")
                    nc.gpsimd.tensor_mul(cxt[:], c2qt[:], cnat[b][:, m, :])
                    nc.sync.dma_start(o_cc2q[b, m * P:(m + 1) * P, :], cxt[:])

            # ---- phase 3: Z -> 1/Z, S2T = E * (1/Z), W = S2T^T@c, Q2C ----
            # Z loads via HWDGE (keeps the Pool engine free for the S2T
            # multiplies); the bf16->fp32 widening runs on the idle ACT.
            Z = []
            for m in range(LB):
                zb = stp.tile([P, L], BF16, name=f"Zb{m}", tag="zb", bufs=2)
                nc.sync.dma_start(zb[:], zout[m * P:(m + 1) * P, :])
                zt = bigp.tile([P, L], F32, name=f"Z{m}", tag="big")
                nc.scalar.copy(zt[:], zb[:])
                nc.vector.reciprocal_approx_fast(out=zt[:], in_=zt[:])
                Z.append(zt)

            # S2T in place of E (E dead after this phase).  b1 goes on
            # gpsimd trailing the DVE recips so GEMM3(b1) can start first;
            # b0 follows on DVE.
            for m in range(LB):
                nc.gpsimd.tensor_mul(_r(E[1][m][:]), E[1][m][:], Z[m][:])
            for m in range(LB):
                nc.vector.tensor_mul(_r(E[0][m][:]), E[0][m][:], Z[m][:])
            # NOTE: one PSUM accumulation group per bank (start=True clears
            # the whole bank's has_written bits), so W and Q2C tiles each
            # get a bank-padded psS slot.  GEMM order b1,b0 for GEMM3 then
            # b1,b0 for GEMM4: each batch's last W evac hides under the
            # other batch's GEMM3.
            for b in (1, 0):
                W = []
                for jm in range(LB):
                    ps = psS.tile([P, D], F32, name="psw", tag="ps")
                    for ik in range(LB):
                        nc.tensor.matmul(
                            ps[:], _r(E[b][ik][:, jm * P:(jm + 1) * P]),
                            _r(cnat[b][:, ik, :]),
                            start=(ik == 0), stop=(ik == LB - 1),
                        )
                    wt = Wp.tile([P, D], BF16, name=f"W{b}_{jm}", tag="W")
                    nc.scalar.copy(wt[:], ps[:])
                    W.append(wt)
                for m in range(LB):
                    # psT's bank-padded slots are idle after the transposes;
                    # using them here decouples the GEMM4 stream from GEMM3's
                    # psS rotation.
                    ps = psT.tile([P, D], F32, name="psq", tag="pt")
                    for jk in range(LB):
                        nc.tensor.matmul(
                            ps[:], E1T[b][jk][:, m * P:(m + 1) * P], W[jk][:],
                            start=(jk == 0), stop=(jk == LB - 1),
                        )
                    q2ct = stp.tile([P, D], F32, name="q2ct", tag="c2q")
                    nc.scalar.activation(
                        q2ct[:], ps[:], AF.Copy, bias=0.0,
                        scale=rD1[b][:, m:m + 1],
                    )
                    cx2t = stp.tile([P, D], F32, name="cx2t", tag="cx")
                    nc.vector.tensor_mul(cx2t[:], q2ct[:], cnat[b][:, m, :])
                    nc.sync.dma_start(o_cq2c[b, m * P:(m + 1) * P, :], cx2t[:])

    nc.compile()  # Bacc defers register allocation; walrus needs it done
    return nc


def _get_nc():
    if "nc" not in _CACHE:
        _CACHE["nc"] = _build_nc()
    return _CACHE["nc"]


def kernel(c, q, c_mask=None, q_mask=None, c_weight=None, q_weight=None,
           cq_weight=None, bias=None, _trace=False, **_ignored):
    c = np.ascontiguousarray(np.asarray(c, dtype=np.float32))
    q = np.ascontiguousarray(np.asarray(q, dtype=np.float32))
    c_weight = np.asarray(c_weight, dtype=np.float32).reshape(D, 1)
    q_weight = np.asarray(q_weight, dtype=np.float32).reshape(D, 1)
    cq_weight = np.asarray(cq_weight, dtype=np.float32).reshape(D)

    # Host-side tiny GEMVs (8 MFLOP; the device kernel does the ~34 GFLOP part).
    s0 = (c @ c_weight)[:, :, 0]  # [B, L]
    s1 = (c @ q_weight)[:, :, 0]  # [B, L]
    # column layout [128, LB] (partition p of block m holds index m*128+p)
    s0c = np.ascontiguousarray(s0.reshape(B, LB, P).transpose(0, 2, 1))
    s1c = np.ascontiguousarray(s1.reshape(B, LB, P).transpose(0, 2, 1))
    cqw = np.ascontiguousarray(cq_weight.reshape(DB, P).T)  # [128, 2]

    nc = _get_nc()
    in_maps = []
    for k in range(NCORES):
        sl = slice(k * BPC, (k + 1) * BPC)
        in_maps.append({
            "c2": c[sl],
            "q2": q[sl],
            "s0c": np.ascontiguousarray(s0c[sl]),
            "s1c": np.ascontiguousarray(s1c[sl]),
            "s1r": np.ascontiguousarray(s1[sl]),
            "cqw": cqw,
        })

    res = run_bass_kernel_spmd(
        nc, in_maps, core_ids=list(range(NCORES)), trace=_trace
    )
    _CACHE["last_result"] = res

    out = np.empty((4 * B, L, D), dtype=np.float32)
    out[0:B] = c
    for k in range(NCORES):
        sl = slice(k * BPC, (k + 1) * BPC)
        r = res.results[k]
        out[B:2 * B][sl] = r["o_c2q"]
        out[2 * B:3 * B][sl] = r["o_cc2q"]
        out[3 * B:4 * B][sl] = r["o_cq2c"]
    return out



# revision 20
# speedup vs baseline: 1.2616x; 1.2616x over previous
"""Trainium2 Bass kernel for the BiAttention problem.

Math (per batch b, L=1024, D=256):
  s0[i] = sum_d c[i,d] * c_weight[d]
  s1[j] = sum_d c[j,d] * q_weight[d]
  s2[i,j] = sum_d (c[i,d]*cqw[d]) * q[j,d]
  S = s0 + s1 + s2 (+bias; scalar bias cancels in both softmaxes)
  S1 = softmax_j(S)
  C2Q = S1 @ q
  S2[b,j,i] = softmax over b of S[b,i,j]  (cross-batch -> AllReduce of exp-sums)
  Q2C = S1 @ (S2 @ c)       (re-associated from (S1@S2)@c)
  out = concat(c, C2Q, c*C2Q, c*Q2C) on axis 0.

Sharding: batch B=16 over 8 cores (2 per core).  The only cross-core
data is Z[i,j] = sum_b exp(S[b,i,j]) -> one fp16 [1024,1024] AllReduce.

Implementation notes:
  - All bulk operands fp16 (full-rate on PE; whole-chain error ~8e-4
    absmax-relative in a numpy bit-model, fits the 2e-2 gate).
  - (c*cqw)^T and q^T are pre-transposed on the HOST and DMA'd directly:
    no input PE-transposes, phase 1 starts as soon as chunks land.
  - E^T (stationary for the S1 GEMMs) comes from PE-transposing the
    phase-1 E tiles inside the AllReduce window (vs recomputing the
    transposed s2 GEMM).
  - E(b0)+E(b1) summed on DVE, one zin write per row-block on HWDGE:
    the AllReduce starts right after phase 1 drains.
  - cn/qn loads sit on the sync HWDGE queue BEHIND the zin writes, so
    their SEQ-level sem waits defer the transfers until phase-1 bulk
    traffic is done (they are only needed mid-AllReduce).
  - W GEMM is ik-major across 8 bank-padded PSUM groups so it starts on
    the first S2T tile instead of the last.
  - Outputs staged in SBUF, written as a few large fp16 DMAs (HWDGE
    descriptor overhead, not bytes, dominated the old per-block writes).
"""

import sys

import numpy as np

for _p in ("/opt/trn_rl_repo",):
    if _p not in sys.path:
        sys.path.insert(0, _p)

import concourse.bacc as bacc
import concourse.bass as bass
import concourse.mybir as mybir
import concourse.tile as tile
from concourse.bass_utils import run_bass_kernel_spmd
from concourse.masks import make_identity

F32 = mybir.dt.float32
F16 = mybir.dt.float16
AF = mybir.ActivationFunctionType
ALU = mybir.AluOpType

B, L, D = 16, 1024, 256
NCORES = 8
BPC = B // NCORES  # batches per core
P = 128
LB = L // P   # 8 L-blocks
DB = D // P   # 2 D-chunks

_CACHE = {}


def _build_nc():
    nc = bacc.Bacc(
        "TRN2",
        target_bir_lowering=False,
        debug=False,
        num_devices=NCORES,
    )

    # ---- kernel I/O (bulk fp16; s0 fp32 for the exp bias) ----
    at_d = nc.dram_tensor("at2", [BPC, DB, P, L], F16, kind="ExternalInput")
    qt_d = nc.dram_tensor("qt2", [BPC, DB, P, L], F16, kind="ExternalInput")
    cn_d = nc.dram_tensor("cn2", [BPC, L, D], F16, kind="ExternalInput")
    qn_d = nc.dram_tensor("qn2", [BPC, L, D], F16, kind="ExternalInput")
    s0c_d = nc.dram_tensor("s0c", [BPC, P, LB], F32, kind="ExternalInput")
    s1r_d = nc.dram_tensor("s1r", [BPC, L], F16, kind="ExternalInput")

    o_c2q = nc.dram_tensor("o_c2q", [BPC, L, D], F16, kind="ExternalOutput")
    o_cc2q = nc.dram_tensor("o_cc2q", [BPC, L, D], F16, kind="ExternalOutput")
    o_cq2c = nc.dram_tensor("o_cq2c", [BPC, L, D], F16, kind="ExternalOutput")

    rg = [list(range(NCORES))]

    with tile.TileContext(nc) as tc:
        with (
            tc.tile_pool(name="dram", bufs=1, space="DRAM") as dram,
            tc.tile_pool(name="small", bufs=1) as small,
            tc.tile_pool(name="atp", bufs=1) as atp,
            tc.tile_pool(name="qtp", bufs=1) as qtp,
            tc.tile_pool(name="cnp", bufs=1) as cnp,
            tc.tile_pool(name="qnp", bufs=1) as qnp,
            tc.tile_pool(name="Ep", bufs=16) as Ep,
            tc.tile_pool(name="Esp", bufs=2) as Esp,
            tc.tile_pool(name="E1Tp", bufs=16) as E1Tp,
            tc.tile_pool(name="Wp", bufs=16) as Wp,
            tc.tile_pool(name="Zp", bufs=8) as Zp,
            tc.tile_pool(name="stg", bufs=1) as stg,
            tc.tile_pool(name="st", bufs=4) as stp,
        ):
            zin = dram.tile([L, L], F16, name="zin")
            zout = dram.tile([L, L], F16, name="zout", addr_space="Shared")

            # ---- constants / small vectors ----
            ident = small.tile([P, P], F16, name="ident")
            make_identity(nc, ident)
            ones16 = small.tile([1, P], F16, name="ones16")
            nc.gpsimd.memset(ones16[:], 1.0)
            s0c = [small.tile([P, LB], F32, name=f"s0c{b}") for b in range(BPC)]
            for b in range(BPC):
                nc.scalar.dma_start(s0c[b][:], s0c_d[b])
            s1r = small.tile([1, BPC * L], F16, name="s1r")
            nc.scalar.dma_start(s1r[:], s1r_d.rearrange("b l -> (b l)")[None, :])
            rsE = [small.tile([P, LB], F32, name=f"rsE{b}") for b in range(BPC)]
            rD1 = [small.tile([P, LB], F32, name=f"rD1{b}") for b in range(BPC)]

            # ---- phase-1-critical loads: one DMA per operand per batch,
            # b0 first, on the sync (SP) HWDGE queue ----
            AT, qT = [None] * BPC, [None] * BPC
            for b in range(BPC):
                att = atp.tile([P, DB, L], F16, name=f"AT{b}")
                qtt = qtp.tile([P, DB, L], F16, name=f"qT{b}")
                for t in range(DB):
                    nc.sync.dma_start(att[:, t], at_d[b, t])
                    nc.sync.dma_start(qtt[:, t], qt_d[b, t])
                AT[b] = att
                qT[b] = qtt

            # ---- phase 1: pv = s2 + s1 (rank-1), E = exp(pv + s0) fp16 ----
            # emission order interleaves the batches once b1's operands have
            # landed, so the ACT exp stream tracks the matmul stream and the
            # last zin block drains right behind the last matmul.
            PH1_ORDER = [(0, 0), (0, 1), (0, 2), (0, 3),
                         (1, 0), (0, 4), (1, 1), (0, 5), (1, 2), (0, 6),
                         (1, 3), (0, 7), (1, 4), (1, 5), (1, 6), (1, 7)]
            E = [[None] * LB for _ in range(BPC)]
            with tc.tile_pool(name="psV", bufs=2, space="PSUM") as psV:
                for b, m in PH1_ORDER:
                    pv = psV.tile([P, L], F32, name="pv", tag="pv")
                    for n in range(2):
                        sl = slice(n * 512, (n + 1) * 512)
                        nc.tensor.matmul(
                            pv[:, sl], AT[b][:, 0, m * P:(m + 1) * P],
                            qT[b][:, 0, sl], start=True, stop=False,
                        )
                        nc.tensor.matmul(
                            pv[:, sl], AT[b][:, 1, m * P:(m + 1) * P],
                            qT[b][:, 1, sl], start=False, stop=False,
                        )
                        nc.tensor.matmul(
                            pv[:, sl], ones16[0:1, :],
                            s1r[0:1, b * L + n * 512: b * L + (n + 1) * 512],
                            start=False, stop=True,
                        )
                    E[b][m] = Ep.tile([P, L], F16, name=f"E{b}_{m}", tag="E")
                    nc.scalar.activation(
                        E[b][m][:], pv[:], AF.Exp,
                        bias=s0c[b][:, m:m + 1],
                        accum_out=rsE[b][:, m:m + 1],
                    )
                    if b == 1:
                        es = Esp.tile([P, L], F16, name="esum", tag="es")
                        nc.vector.tensor_tensor(
                            out=es[:], in0=E[0][m][:], in1=E[1][m][:],
                            op=ALU.add,
                        )
                        nc.sync.dma_start(zin[m * P:(m + 1) * P, :], es[:])
                for b in range(BPC):
                    nc.vector.reciprocal(out=rD1[b][:], in_=rsE[b][:])

            # ---- cross-batch softmax denominator AllReduce ----
            nc.gpsimd.collective_compute(
                "AllReduce", ALU.add, replica_groups=rg,
                ins=[zin.opt()], outs=[zout.opt()],
            )

            # cn/qn sit on the sync queue BEHIND the zin writes: the queue's
            # in-order SEQ sem-waits defer these transfers until phase 1 has
            # drained, keeping the DMA engines clear for the critical loads.
            cnat, qnat = [], []
            for b in range(BPC):
                qt_ = qnp.tile([P, LB, D], F16, name=f"qn{b}")
                nc.sync.dma_start(qt_[:], qn_d[b].rearrange("(m p) d -> p m d", p=P))
                qnat.append(qt_)
                ct = cnp.tile([P, LB, D], F16, name=f"cn{b}")
                nc.sync.dma_start(ct[:], cn_d[b].rearrange("(m p) d -> p m d", p=P))
                cnat.append(ct)

            # output staging (fp16, written once per tensor per batch)
            sg_c2q = [stg.tile([P, LB, D], F16, name=f"sgA{b}") for b in range(BPC)]
            sg_cc2q = [stg.tile([P, LB, D], F16, name=f"sgB{b}") for b in range(BPC)]
            sg_cq2c = [stg.tile([P, LB, D], F16, name=f"sgC{b}") for b in range(BPC)]

            # ---- helpers for the GEMM phases ----
            E1T = [[None] * LB for _ in range(BPC)]

            def c2q_group(psC, b, m):
                """One C2Q accumulation group + evac + c*C2Q mul."""
                ps = psC.tile([P, D], F32, name="psc", tag="psc")
                for jk in range(LB):
                    nc.tensor.matmul(
                        ps[:], E1T[b][jk][:, m * P:(m + 1) * P],
                        qnat[b][:, jk, :],
                        start=(jk == 0), stop=(jk == LB - 1),
                    )
                nc.scalar.activation(
                    sg_c2q[b][:, m, :], ps[:], AF.Copy, bias=0.0,
                    scale=rD1[b][:, m:m + 1],
                )
                nc.vector.tensor_mul(
                    sg_cc2q[b][:, m, :], sg_c2q[b][:, m, :], cnat[b][:, m, :]
                )

            W = [[None] * LB for _ in range(BPC)]

            def w_evac(psw, b, jm):
                wt = Wp.tile([P, D], F16, name=f"W{b}_{jm}", tag="W")
                if jm % 2 == 0:
                    nc.vector.tensor_copy(out=wt[:], in_=psw[:])
                else:
                    nc.scalar.copy(wt[:], psw[:])
                W[b][jm] = wt

            def q2c_group(pool, b, m, **tkw):
                """One Q2C accumulation group + evac + c*Q2C mul."""
                ps = pool.tile([P, D], F32, name="psq", **tkw)
                for jk in range(LB):
                    nc.tensor.matmul(
                        ps[:], E1T[b][jk][:, m * P:(m + 1) * P], W[b][jk][:],
                        start=(jk == 0), stop=(jk == LB - 1),
                    )
                q2ct = stp.tile([P, D], F16, name="q2ct", tag="q2c")
                nc.scalar.activation(
                    q2ct[:], ps[:], AF.Copy, bias=0.0,
                    scale=rD1[b][:, m:m + 1],
                )
                nc.vector.tensor_mul(
                    sg_cq2c[b][:, m, :], q2ct[:], cnat[b][:, m, :]
                )

            # psC: ONE half-bank pair for all (sequential) C2Q/Q2C groups.
            # psW: 7 exclusive banks for the concurrently-accumulating
            # ik-major W groups (jm0-6); jm7 trails jm-major and reuses the
            # pool rotation (waits jm0's evac via WAR dep).
            with tc.tile_pool(name="psC", bufs=1, space="PSUM") as psC:
                # ---- AR window: E1T = E^T via PE transposes (evacs on the
                # otherwise-idle ACT); C2Q-b0 ----
                with tc.tile_pool(name="psTr", bufs=4, space="PSUM") as psTr:
                    for b in range(BPC):
                        for jm in range(LB):
                            ptr = psTr.tile([P, L], F16, name="ptr", tag="ptr")
                            for m in range(LB):
                                nc.tensor.transpose(
                                    ptr[:, m * P:(m + 1) * P],
                                    E[b][m][:, jm * P:(jm + 1) * P],
                                    ident[:],
                                )
                            e1t = E1Tp.tile([P, L], F16, name=f"E1T{b}_{jm}",
                                            tag="E1T")
                            nc.vector.tensor_copy(out=e1t[:], in_=ptr[:])
                            E1T[b][jm] = e1t
                with tc.tile_pool(name="psC0", bufs=3, space="PSUM") as psC0:
                    for m in range(LB):
                        c2q_group(psC0, 0, m)
                nc.scalar.dma_start(
                    o_c2q[0].rearrange("(m p) d -> p m d", p=P), sg_c2q[0][:]
                )
                nc.scalar.dma_start(
                    o_cc2q[0].rearrange("(m p) d -> p m d", p=P), sg_cc2q[0][:]
                )

                # ---- Z chain: zb load -> widen (ACT) -> approx recip (DVE)
                # -> S2T muls (b1 split DVE/Pool first, then b0) ----
                # mul split 3 DVE / 5 Pool per batch balances
                # DVE(recips+muls) vs Pool(muls) production spans.
                zts = []
                B1_DVE = {0, 3, 6}
                for m in range(LB):
                    zb = stp.tile([P, L], F16, name=f"zb{m}", tag="zb", bufs=3)
                    nc.sync.dma_start(zb[:], zout[m * P:(m + 1) * P, :])
                    zt = Zp.tile([P, L], F32, name=f"Z{m}", tag="Z")
                    nc.scalar.copy(zt[:], zb[:])
                    nc.vector.reciprocal_approx_fast(out=zt[:], in_=zt[:])
                    if m in B1_DVE:
                        nc.vector.tensor_mul(E[1][m][:], E[1][m][:], zt[:])
                    else:
                        nc.gpsimd.tensor_mul(E[1][m][:], E[1][m][:], zt[:])
                    zts.append(zt)
                for m in range(LB):
                    if m in B1_DVE:
                        nc.gpsimd.tensor_mul(E[0][m][:], E[0][m][:], zts[m][:])
                    else:
                        nc.vector.tensor_mul(E[0][m][:], E[0][m][:], zts[m][:])

                with tc.tile_pool(name="psW", bufs=7, space="PSUM") as psW:
                    for b in (1, 0):
                        # interleave: per slot one C2Q/Q2C group (keeps PE
                        # streaming at production cadence) + W ik-step for
                        # jm0-6; jm7 runs jm-major once all S2T are out.
                        psw = [
                            psW.tile([P, D], F32, name=f"psw{b}_{jm}",
                                     tag="psw", padded_shape=[P, 512])
                            for jm in range(7)
                        ]
                        for k in range(LB):
                            if b == 1:
                                c2q_group(psC, 1, k)
                            else:
                                q2c_group(psC, 1, k, tag="psc")
                            for jm in range(7):
                                nc.tensor.matmul(
                                    psw[jm][:],
                                    E[b][k][:, jm * P:(jm + 1) * P],
                                    cnat[b][:, k, :],
                                    start=(k == 0), stop=(k == LB - 1),
                                )
                        for jm in range(7):
                            w_evac(psw[jm], b, jm)
                        psw7 = psW.tile([P, D], F32, name=f"psw{b}_7",
                                        tag="psw", padded_shape=[P, 512])
                        for ik in range(LB):
                            nc.tensor.matmul(
                                psw7[:], E[b][ik][:, 7 * P:8 * P],
                                cnat[b][:, ik, :],
                                start=(ik == 0), stop=(ik == LB - 1),
                            )
                        w_evac(psw7, b, 7)
                        if b == 1:
                            nc.scalar.dma_start(
                                o_c2q[1].rearrange("(m p) d -> p m d", p=P),
                                sg_c2q[1][:],
                            )
                            nc.scalar.dma_start(
                                o_cc2q[1].rearrange("(m p) d -> p m d", p=P),
                                sg_cc2q[1][:],
                            )
                        else:
                            nc.scalar.dma_start(
                                o_cq2c[1].rearrange("(m p) d -> p m d", p=P),
                                sg_cq2c[1][:],
                            )

                    # final Q2C (b0) on the freed psW banks (own bank per
                    # group: no evac serialization), output in halves
                    for m in range(LB):
                        q2c_group(psW, 0, m, tag="psw",
                                  padded_shape=[P, 512])
                        if m == LB // 2 - 1:
                            nc.scalar.dma_start(
                                o_cq2c[0, 0:L // 2].rearrange(
                                    "(m p) d -> p m d", p=P),
                                sg_cq2c[0][:, 0:LB // 2, :],
                            )
                    nc.scalar.dma_start(
                        o_cq2c[0, L // 2:L].rearrange("(m p) d -> p m d", p=P),
                        sg_cq2c[0][:, LB // 2:LB, :],
                    )

    nc.compile()
    return nc


def _get_nc():
    if "nc" not in _CACHE:
        _CACHE["nc"] = _build_nc()
    return _CACHE["nc"]


def kernel(c, q, c_mask=None, q_mask=None, c_weight=None, q_weight=None,
           cq_weight=None, bias=None, _trace=False, **_ignored):
    c = np.ascontiguousarray(np.asarray(c, dtype=np.float32))
    q = np.ascontiguousarray(np.asarray(q, dtype=np.float32))
    c_weight = np.asarray(c_weight, dtype=np.float32).reshape(D, 1)
    q_weight = np.asarray(q_weight, dtype=np.float32).reshape(D, 1)
    cq_weight = np.asarray(cq_weight, dtype=np.float32).reshape(D)

    # Host-side tiny GEMVs (8 MFLOP) + fp16 casts / transposes.
    s0 = (c @ c_weight)[:, :, 0]  # [B, L]
    s1 = (c @ q_weight)[:, :, 0]  # [B, L]
    s0c = np.ascontiguousarray(s0.reshape(B, LB, P).transpose(0, 2, 1))
    s1r = s1.astype(np.float16)
    A = (c * cq_weight).astype(np.float16)
    at = np.ascontiguousarray(A.transpose(0, 2, 1)).reshape(B, DB, P, L)
    q16 = q.astype(np.float16)
    qt = np.ascontiguousarray(q16.transpose(0, 2, 1)).reshape(B, DB, P, L)
    c16 = c.astype(np.float16)

    nc = _get_nc()
    in_maps = []
    for k in range(NCORES):
        sl = slice(k * BPC, (k + 1) * BPC)
        in_maps.append({
            "at2": at[sl],
            "qt2": qt[sl],
            "cn2": c16[sl],
            "qn2": q16[sl],
            "s0c": np.ascontiguousarray(s0c[sl]),
            "s1r": np.ascontiguousarray(s1r[sl]),
        })

    res = run_bass_kernel_spmd(
        nc, in_maps, core_ids=list(range(NCORES)), trace=_trace
    )
    _CACHE["last_result"] = res

    out = np.empty((4 * B, L, D), dtype=np.float32)
    out[0:B] = c
    for k in range(NCORES):
        sl = slice(k * BPC, (k + 1) * BPC)
        r = res.results[k]
        out[B:2 * B][sl] = r["o_c2q"].astype(np.float32)
        out[2 * B:3 * B][sl] = r["o_cc2q"].astype(np.float32)
        out[3 * B:4 * B][sl] = r["o_cq2c"].astype(np.float32)
    return out


# revision 28
# speedup vs baseline: 1.2743x; 1.0100x over previous
"""Trainium2 Bass kernel for the BiAttention problem.

Math (per batch b, L=1024, D=256):
  s0[i] = sum_d c[i,d] * c_weight[d]
  s1[j] = sum_d c[j,d] * q_weight[d]
  s2[i,j] = sum_d (c[i,d]*cqw[d]) * q[j,d]
  S = s0 + s1 + s2 (+bias; scalar bias cancels in both softmaxes)
  S1 = softmax_j(S)
  C2Q = S1 @ q
  S2[b,j,i] = softmax over b of S[b,i,j]  (cross-batch -> AllReduce of exp-sums)
  Q2C = S1 @ (S2 @ c)       (re-associated from (S1@S2)@c)
  out = concat(c, C2Q, c*C2Q, c*Q2C) on axis 0.

Sharding: batch B=16 over 8 cores (2 per core).  The only cross-core
data is Z[i,j] = sum_b exp(S[b,i,j]) -> one fp16 [1024,1024] AllReduce.

Implementation notes:
  - All bulk operands fp16 (full-rate on PE; whole-chain error ~8e-4
    absmax-relative in a numpy bit-model, fits the 2e-2 gate).
  - (c*cqw)^T and q^T are pre-transposed on the HOST and DMA'd directly:
    no input PE-transposes, phase 1 starts as soon as chunks land.
  - E^T (stationary for the S1 GEMMs) comes from PE-transposing the
    phase-1 E tiles inside the AllReduce window (vs recomputing the
    transposed s2 GEMM).
  - E(b0)+E(b1) summed on DVE, one zin write per row-block on HWDGE:
    the AllReduce starts right after phase 1 drains.
  - cn/qn loads sit on the sync HWDGE queue BEHIND the zin writes, so
    their SEQ-level sem waits defer the transfers until phase-1 bulk
    traffic is done (they are only needed mid-AllReduce).
  - W GEMM is ik-major across 8 bank-padded PSUM groups so it starts on
    the first S2T tile instead of the last.
  - Outputs staged in SBUF, written as a few large fp16 DMAs (HWDGE
    descriptor overhead, not bytes, dominated the old per-block writes).
"""

import sys

import numpy as np

for _p in ("/opt/trn_rl_repo",):
    if _p not in sys.path:
        sys.path.insert(0, _p)

import concourse.bacc as bacc
import concourse.bass as bass
import concourse.mybir as mybir
import concourse.tile as tile
from concourse.bass_utils import run_bass_kernel_spmd
from concourse.masks import make_identity

F32 = mybir.dt.float32
F16 = mybir.dt.float16
F8 = mybir.dt.float8e4
AF = mybir.ActivationFunctionType
ALU = mybir.AluOpType
DR = mybir.MatmulPerfMode.DoubleRow

B, L, D = 16, 1024, 256
NCORES = 8
BPC = B // NCORES  # batches per core
P = 128
LB = L // P   # 8 L-blocks
DB = D // P   # 2 D-chunks

_CACHE = {}


def _build_nc():
    nc = bacc.Bacc(
        "TRN2",
        target_bir_lowering=False,
        debug=False,
        num_devices=NCORES,
    )

    # ---- kernel I/O (bulk fp16; s0 fp32 for the exp bias) ----
    at_d = nc.dram_tensor("at2", [BPC, DB, P, L], F16, kind="ExternalInput")
    qt_d = nc.dram_tensor("qt2", [BPC, DB, P, L], F16, kind="ExternalInput")
    cn_d = nc.dram_tensor("cn2", [BPC, L, D], F16, kind="ExternalInput")
    qn_d = nc.dram_tensor("qn2", [BPC, L, D], F16, kind="ExternalInput")
    s0c_d = nc.dram_tensor("s0c", [BPC, P, LB], F32, kind="ExternalInput")
    # s1 split into fp8 hi+lo rows for the DoubleRow rank-1 (hi+lo
    # reconstructs s1 to ~0.3%, below the fp16 operand noise floor)
    s1p_d = nc.dram_tensor("s1p", [BPC, 2, L], F8, kind="ExternalInput")

    o_c2q = nc.dram_tensor("o_c2q", [BPC, L, D], F16, kind="ExternalOutput")
    o_cc2q = nc.dram_tensor("o_cc2q", [BPC, L, D], F16, kind="ExternalOutput")
    o_cq2c = nc.dram_tensor("o_cq2c", [BPC, L, D], F16, kind="ExternalOutput")

    rg = [list(range(NCORES))]

    with tile.TileContext(nc) as tc:
        with (
            tc.tile_pool(name="dram", bufs=1, space="DRAM") as dram,
            tc.tile_pool(name="small", bufs=1) as small,
            tc.tile_pool(name="atp", bufs=1) as atp,
            tc.tile_pool(name="qtp", bufs=1) as qtp,
            tc.tile_pool(name="cnp", bufs=1) as cnp,
            tc.tile_pool(name="qnp", bufs=1) as qnp,
            tc.tile_pool(name="Ep", bufs=16) as Ep,
            tc.tile_pool(name="Esp", bufs=2) as Esp,
            tc.tile_pool(name="E1Tp", bufs=16) as E1Tp,
            tc.tile_pool(name="Wp", bufs=16) as Wp,
            tc.tile_pool(name="Zp", bufs=8) as Zp,
            tc.tile_pool(name="stg", bufs=1) as stg,
            tc.tile_pool(name="st", bufs=4) as stp,
        ):
            zin = dram.tile([L, L], F16, name="zin")
            zout = dram.tile([L, L], F16, name="zout", addr_space="Shared")

            # ---- constants / small vectors ----
            ident = small.tile([P, P], F16, name="ident")
            make_identity(nc, ident)
            ones8 = small.tile([1, 2, P], F8, name="ones8")
            nc.gpsimd.memset(ones8[:], 1.0)
            s0c = [small.tile([P, LB], F32, name=f"s0c{b}") for b in range(BPC)]
            for b in range(BPC):
                nc.scalar.dma_start(s0c[b][:], s0c_d[b])
            s1p = small.tile([1, BPC, 2, L], F8, name="s1p")
            nc.scalar.dma_start(
                s1p[:], s1p_d.rearrange("b t l -> (b t l)")[None, :])
            rsE = [small.tile([P, LB], F32, name=f"rsE{b}") for b in range(BPC)]
            rD1 = [small.tile([P, LB], F32, name=f"rD1{b}") for b in range(BPC)]

            # ---- phase-1-critical loads: one DMA per operand per batch,
            # b0 first, on the sync (SP) HWDGE queue ----
            AT, qT = [None] * BPC, [None] * BPC
            for b in range(BPC):
                att = atp.tile([P, DB, L], F16, name=f"AT{b}")
                qtt = qtp.tile([P, DB, L], F16, name=f"qT{b}")
                for t in range(DB):
                    nc.sync.dma_start(att[:, t], at_d[b, t])
                    nc.sync.dma_start(qtt[:, t], qt_d[b, t])
                AT[b] = att
                qT[b] = qtt

            # ---- phase 1: pv = s2 + s1 (rank-1), E = exp(pv + s0) fp16 ----
            # emission order interleaves the batches once b1's operands have
            # landed, so the ACT exp stream tracks the matmul stream and the
            # last zin block drains right behind the last matmul.
            PH1_ORDER = [(0, 0), (0, 1), (0, 2), (0, 3),
                         (1, 0), (0, 4), (1, 1), (0, 5), (1, 2), (0, 6),
                         (1, 3), (0, 7), (1, 4), (1, 5), (1, 6), (1, 7)]
            E = [[None] * LB for _ in range(BPC)]
            with tc.tile_pool(name="psV", bufs=2, space="PSUM") as psV:
                for b, m in PH1_ORDER:
                    pv = psV.tile([P, L], F32, name="pv", tag="pv")
                    for n in range(2):
                        sl = slice(n * 512, (n + 1) * 512)
                        nc.tensor.matmul(
                            pv[:, sl], AT[b][:, 0, m * P:(m + 1) * P],
                            qT[b][:, 0, sl], start=True, stop=False,
                        )
                        nc.tensor.matmul(
                            pv[:, sl], AT[b][:, 1, m * P:(m + 1) * P],
                            qT[b][:, 1, sl], start=False, stop=False,
                        )
                        nc.tensor.matmul(
                            pv[:, sl], ones8[0:1, :, :], s1p[0:1, b, :, sl],
                            start=False, stop=True, perf_mode=DR,
                        )
                    E[b][m] = Ep.tile([P, L], F16, name=f"E{b}_{m}", tag="E")
                    nc.scalar.activation(
                        E[b][m][:], pv[:], AF.Exp,
                        bias=s0c[b][:, m:m + 1],
                        accum_out=rsE[b][:, m:m + 1],
                    )
                    if b == 1:
                        es = Esp.tile([P, L], F16, name="esum", tag="es")
                        nc.vector.tensor_tensor(
                            out=es[:], in0=E[0][m][:], in1=E[1][m][:],
                            op=ALU.add,
                        )
                        nc.sync.dma_start(zin[m * P:(m + 1) * P, :], es[:])
                for b in range(BPC):
                    nc.vector.reciprocal(out=rD1[b][:], in_=rsE[b][:])

            # ---- cross-batch softmax denominator AllReduce ----
            nc.gpsimd.collective_compute(
                "AllReduce", ALU.add, replica_groups=rg,
                ins=[zin.opt()], outs=[zout.opt()],
            )

            # cn/qn sit on the sync queue BEHIND the zin writes: the queue's
            # in-order SEQ sem-waits defer these transfers until phase 1 has
            # drained, keeping the DMA engines clear for the critical loads.
            cnat, qnat = [], []
            for b in range(BPC):
                qt_ = qnp.tile([P, LB, D], F16, name=f"qn{b}")
                nc.sync.dma_start(qt_[:], qn_d[b].rearrange("(m p) d -> p m d", p=P))
                qnat.append(qt_)
                ct = cnp.tile([P, LB, D], F16, name=f"cn{b}")
                nc.sync.dma_start(ct[:], cn_d[b].rearrange("(m p) d -> p m d", p=P))
                cnat.append(ct)

            # output staging (fp16, written once per tensor per batch)
            sg_c2q = [stg.tile([P, LB, D], F16, name=f"sgA{b}") for b in range(BPC)]
            sg_cc2q = [stg.tile([P, LB, D], F16, name=f"sgB{b}") for b in range(BPC)]
            sg_cq2c = [stg.tile([P, LB, D], F16, name=f"sgC{b}") for b in range(BPC)]

            # ---- helpers for the GEMM phases ----
            E1T = [[None] * LB for _ in range(BPC)]

            def c2q_group(psC, b, m):
                """One C2Q accumulation group + evac + c*C2Q mul."""
                ps = psC.tile([P, D], F32, name="psc", tag="psc")
                for jk in range(LB):
                    nc.tensor.matmul(
                        ps[:], E1T[b][jk][:, m * P:(m + 1) * P],
                        qnat[b][:, jk, :],
                        start=(jk == 0), stop=(jk == LB - 1),
                    )
                nc.scalar.activation(
                    sg_c2q[b][:, m, :], ps[:], AF.Copy, bias=0.0,
                    scale=rD1[b][:, m:m + 1],
                )
                nc.vector.tensor_mul(
                    sg_cc2q[b][:, m, :], sg_c2q[b][:, m, :], cnat[b][:, m, :]
                )

            W = [[None] * LB for _ in range(BPC)]

            def w_evac(psw, b, jm):
                wt = Wp.tile([P, D], F16, name=f"W{b}_{jm}", tag="W")
                if jm % 2 == 0:
                    nc.vector.tensor_copy(out=wt[:], in_=psw[:])
                else:
                    nc.scalar.copy(wt[:], psw[:])
                W[b][jm] = wt

            def q2c_group(pool, b, m, **tkw):
                """One Q2C accumulation group + evac + c*Q2C mul."""
                ps = pool.tile([P, D], F32, name="psq", **tkw)
                for jk in range(LB):
                    nc.tensor.matmul(
                        ps[:], E1T[b][jk][:, m * P:(m + 1) * P], W[b][jk][:],
                        start=(jk == 0), stop=(jk == LB - 1),
                    )
                q2ct = stp.tile([P, D], F16, name="q2ct", tag="q2c")
                nc.scalar.activation(
                    q2ct[:], ps[:], AF.Copy, bias=0.0,
                    scale=rD1[b][:, m:m + 1],
                )
                nc.vector.tensor_mul(
                    sg_cq2c[b][:, m, :], q2ct[:], cnat[b][:, m, :]
                )

            # psC: ONE half-bank pair for all (sequential) C2Q/Q2C groups.
            # psW: 7 exclusive banks for the concurrently-accumulating
            # ik-major W groups (jm0-6); jm7 trails jm-major and reuses the
            # pool rotation (waits jm0's evac via WAR dep).
            with tc.tile_pool(name="psC", bufs=1, space="PSUM") as psC:
                # ---- AR window: E1T = E^T via PE transposes (evacs on the
                # otherwise-idle ACT); C2Q-b0 ----
                with tc.tile_pool(name="psTr", bufs=4, space="PSUM") as psTr:
                    for b in range(BPC):
                        for jm in range(LB):
                            ptr = psTr.tile([P, L], F16, name="ptr", tag="ptr")
                            for m in range(LB):
                                nc.tensor.transpose(
                                    ptr[:, m * P:(m + 1) * P],
                                    E[b][m][:, jm * P:(jm + 1) * P],
                                    ident[:],
                                )
                            e1t = E1Tp.tile([P, L], F16, name=f"E1T{b}_{jm}",
                                            tag="E1T")
                            nc.vector.tensor_copy(out=e1t[:], in_=ptr[:])
                            E1T[b][jm] = e1t
                with tc.tile_pool(name="psC0", bufs=3, space="PSUM") as psC0:
                    for m in range(LB):
                        c2q_group(psC0, 0, m)
                nc.scalar.dma_start(
                    o_c2q[0].rearrange("(m p) d -> p m d", p=P), sg_c2q[0][:]
                )
                nc.scalar.dma_start(
                    o_cc2q[0].rearrange("(m p) d -> p m d", p=P), sg_cc2q[0][:]
                )

                # ---- Z chain: zb load -> widen (ACT) -> approx recip (DVE)
                # -> S2T muls (b1 split DVE/Pool first, then b0) ----
                # mul split 3 DVE / 5 Pool per batch balances
                # DVE(recips+muls) vs Pool(muls) production spans.
                zts = []
                B1_DVE = {0, 3, 6}
                for m in range(LB):
                    zt = Zp.tile([P, L], F32, name=f"Z{m}", tag="Z")
                    if m < 2:
                        # chain head: SWDGE casting load (f16->f32 in the DMA)
                        # skips the ACT widen hop; Pool is idle here.
                        nc.gpsimd.dma_start(zt[:], zout[m * P:(m + 1) * P, :])
                    else:
                        zb = stp.tile([P, L], F16, name=f"zb{m}", tag="zb",
                                      bufs=3)
                        nc.sync.dma_start(zb[:], zout[m * P:(m + 1) * P, :])
                        nc.scalar.copy(zt[:], zb[:])
                    nc.vector.reciprocal_approx_fast(out=zt[:], in_=zt[:])
                    if m in B1_DVE:
                        nc.vector.tensor_mul(E[1][m][:], E[1][m][:], zt[:])
                    else:
                        nc.gpsimd.tensor_mul(E[1][m][:], E[1][m][:], zt[:])
                    zts.append(zt)
                for m in range(LB):
                    if m in B1_DVE:
                        nc.gpsimd.tensor_mul(E[0][m][:], E[0][m][:], zts[m][:])
                    else:
                        nc.vector.tensor_mul(E[0][m][:], E[0][m][:], zts[m][:])

                with tc.tile_pool(name="psW", bufs=7, space="PSUM") as psW:
                    for b in (1, 0):
                        # interleave: per slot one C2Q/Q2C group (keeps PE
                        # streaming at production cadence) + W ik-step for
                        # jm0-6; jm7 runs jm-major once all S2T are out.
                        psw = [
                            psW.tile([P, D], F32, name=f"psw{b}_{jm}",
                                     tag="psw", padded_shape=[P, 512])
                            for jm in range(7)
                        ]
                        for k in range(LB):
                            if b == 1:
                                c2q_group(psC, 1, k)
                            else:
                                q2c_group(psC, 1, k, tag="psc")
                            for jm in range(7):
                                nc.tensor.matmul(
                                    psw[jm][:],
                                    E[b][k][:, jm * P:(jm + 1) * P],
                                    cnat[b][:, k, :],
                                    start=(k == 0), stop=(k == LB - 1),
                                )
                        for jm in range(7):
                            w_evac(psw[jm], b, jm)
                        psw7 = psW.tile([P, D], F32, name=f"psw{b}_7",
                                        tag="psw", padded_shape=[P, 512])
                        for ik in range(LB):
                            nc.tensor.matmul(
                                psw7[:], E[b][ik][:, 7 * P:8 * P],
                                cnat[b][:, ik, :],
                                start=(ik == 0), stop=(ik == LB - 1),
                            )
                        w_evac(psw7, b, 7)
                        if b == 1:
                            nc.scalar.dma_start(
                                o_c2q[1].rearrange("(m p) d -> p m d", p=P),
                                sg_c2q[1][:],
                            )
                            nc.scalar.dma_start(
                                o_cc2q[1].rearrange("(m p) d -> p m d", p=P),
                                sg_cc2q[1][:],
                            )
                        else:
                            nc.scalar.dma_start(
                                o_cq2c[1].rearrange("(m p) d -> p m d", p=P),
                                sg_cq2c[1][:],
                            )

                    # final Q2C (b0) on the freed psW banks (own bank per
                    # group: no evac serialization); stream the output so
                    # only the last two row-blocks sit in the drain tail.
                    for m in range(LB):
                        q2c_group(psW, 0, m, tag="psw",
                                  padded_shape=[P, 512])
                        if m in (3, 5):
                            lo = (m - 1) * P if m == 5 else 0
                            nc.scalar.dma_start(
                                o_cq2c[0, lo:(m + 1) * P].rearrange(
                                    "(m p) d -> p m d", p=P),
                                sg_cq2c[0][:, lo // P:m + 1, :],
                            )
                    nc.scalar.dma_start(
                        o_cq2c[0, 6 * P:L].rearrange("(m p) d -> p m d", p=P),
                        sg_cq2c[0][:, 6:LB, :],
                    )

    nc.compile()
    return nc


def _get_nc():
    if "nc" not in _CACHE:
        _CACHE["nc"] = _build_nc()
    return _CACHE["nc"]


def kernel(c, q, c_mask=None, q_mask=None, c_weight=None, q_weight=None,
           cq_weight=None, bias=None, _trace=False, **_ignored):
    c = np.ascontiguousarray(np.asarray(c, dtype=np.float32))
    q = np.ascontiguousarray(np.asarray(q, dtype=np.float32))
    c_weight = np.asarray(c_weight, dtype=np.float32).reshape(D, 1)
    q_weight = np.asarray(q_weight, dtype=np.float32).reshape(D, 1)
    cq_weight = np.asarray(cq_weight, dtype=np.float32).reshape(D)

    # Host-side tiny GEMVs (8 MFLOP) + fp16 casts / transposes.
    s0 = (c @ c_weight)[:, :, 0]  # [B, L]
    s1 = (c @ q_weight)[:, :, 0]  # [B, L]
    s0c = np.ascontiguousarray(s0.reshape(B, LB, P).transpose(0, 2, 1))
    import ml_dtypes
    f8 = ml_dtypes.float8_e4m3fn
    s1hi = s1.astype(f8)
    s1lo = (s1 - s1hi.astype(np.float32)).astype(f8)
    s1p = np.ascontiguousarray(np.stack([s1hi, s1lo], axis=1))  # [B, 2, L]
    A = (c * cq_weight).astype(np.float16)
    at = np.ascontiguousarray(A.transpose(0, 2, 1)).reshape(B, DB, P, L)
    q16 = q.astype(np.float16)
    qt = np.ascontiguousarray(q16.transpose(0, 2, 1)).reshape(B, DB, P, L)
    c16 = c.astype(np.float16)

    nc = _get_nc()
    in_maps = []
    for k in range(NCORES):
        sl = slice(k * BPC, (k + 1) * BPC)
        in_maps.append({
            "at2": at[sl],
            "qt2": qt[sl],
            "cn2": c16[sl],
            "qn2": q16[sl],
            "s0c": np.ascontiguousarray(s0c[sl]),
            "s1p": np.ascontiguousarray(s1p[sl]),
        })

    res = run_bass_kernel_spmd(
        nc, in_maps, core_ids=list(range(NCORES)), trace=_trace
    )
    _CACHE["last_result"] = res

    out = np.empty((4 * B, L, D), dtype=np.float32)
    out[0:B] = c
    for k in range(NCORES):
        sl = slice(k * BPC, (k + 1) * BPC)
        r = res.results[k]
        out[B:2 * B][sl] = r["o_c2q"].astype(np.float32)
        out[2 * B:3 * B][sl] = r["o_cc2q"].astype(np.float32)
        out[3 * B:4 * B][sl] = r["o_cq2c"].astype(np.float32)
    return out
